# revision 21
# baseline (speedup 1.0000x reference)
"""MoE top-2 routing kernel for 8 Trainium2 NeuronCores.

Strategy (expert-parallel with 2-stream load balancing):
  - Host computes the (tiny) router in float64: logits -> softmax -> top-2 ->
    renormalize.  Selection was verified tie-safe: min prob gap between
    2nd/3rd expert is ~8e-6 while cross-backend fp32 logit noise is ~3e-7.
  - The 8192 (token, expert) pairs are packed into 16 capacity units:
    8 "X" units of SX=512 tokens and 8 "Y" units of SY tokens, each unit
    single-expert.  A small solver picks the minimal SY such that every
    expert's token count is covered by an integral number of units
    (experts can span units on different cores).  Each core gets one X
    unit and one Y unit -> per-core capacity C = SX + SY (1048 for the
    staged routing, vs 1088 for plain expert-per-core), with two weight
    streams per core instead of one.
  - Each core runs a dense FFN over its C tokens in nb blocks (Y split
    into <=512-token blocks + the X block), with D/H features on the
    partition axis end-to-end, so no on-device transposes are needed.
        hT = gelu(w1T.T-contractions)   (PSUM fp32 accum, bias fused in ACT)
        oT = w2-contractions over hT
  - Startup is DMA-limited (~100 GB/s per HWDGE queue): the first-needed
    transfers (x block 0, w1Y tiles 0-1) are split across 4 engine queues
    and issued before anything else; PE warm-up matmuls are sized to end
    right when the first real inputs land so HAM stays at 8/8.
  - Host applies combine weights and scatter-adds per-unit outputs back
    into the full [B,S,D] output.

Per-core layouts (D=1024, H=4096, C tokens, P=128):
  xT  [128, 8*C]         bf16   block-major: xT[p, ko*sz+t] = x_blk[t, ko*128+p]
  w1Y/w1X [128, 32, 8, 128] bf16  w1t[p, ho, ko, j] = w1[e][ho*128+j, ko*128+p]
  w2Y/w2X [128, 8, 32, 128] bf16  w2t[p, do, ko, j] = w2[e][do*128+j, ko*128+p]
  b1Y/b1X [128, 32]      f32    b1t[p, ho]         = b1[e][ho*128+p]
  oT  [128, 8, C]        f32    oT[p, do, t]       = o_blk[t, do*128+p]
"""

import numpy as np
import ml_dtypes

TOP_K = 2
P = 128
D = 1024
H = 4096
E = 8
SX = 512
NWARM = 26
WN = 256
LEAD = 3

_COMPILED = {}  # (ybs, sx) -> compiled Bacc instance


def _split_y(SY):
    """Split SY into <=512 blocks (multiples of 8 assumed), smallest first."""
    n = -(-SY // 512)
    base = ((SY // n + 7) // 8) * 8
    first = SY - base * (n - 1)
    assert 0 < first <= base <= 512, (SY, n, base, first)
    return [first] + [base] * (n - 1)


def _try_assign(cnt, SY):
    """Cover counts with 8 X-units (SX) and 8 Y-units (SY); returns
    [(x_e, y_e)] per expert or None."""
    order = sorted(range(E), key=lambda e: -cnt[e])
    res = [None] * E

    def opts(need, xb, yb):
        out = []
        for y in range(min(yb, 8) + 1):
            rem = need - y * SY
            x = max(0, -(-rem // SX)) if rem > 0 else 0
            if x <= xb:
                tot = x * SX + y * SY
                if tot >= need:
                    out.append((tot - need, x, y))
        out.sort()
        # drop dominated (same or more of both units)
        seen = []
        for w, x, y in out:
            if not any(x >= sx and y >= sy for sx, sy in seen):
                seen.append((x, y))
        return seen

    def dfs(i, xb, yb):
        if i == E:
            return True
        e = order[i]
        for x, y in opts(cnt[e], xb, yb):
            res[e] = (x, y)
            if dfs(i + 1, xb - x, yb - y):
                return True
        res[e] = None
        return False

    return res if dfs(0, 8, 8) else None


def _solve_layout(cnt):
    for SY in range(256, 8 * 4096, 8):
        asn = _try_assign(cnt, SY)
        if asn is not None:
            return SY, asn
    raise RuntimeError("no unit assignment found")


def _build_ffn_kernel(ybs, sx):
    import concourse.mybir as mybir
    import concourse.tile as tile
    from concourse import bacc

    ybs = list(ybs)
    nY = len(ybs)
    blocks = ybs + [sx]          # sizes, processing order
    streams = [0] * nY + [1]     # 0 = Y weights, 1 = X weights
    starts = [sum(blocks[:i]) for i in range(len(blocks))]
    nb = len(blocks)
    C = sum(blocks)
    bf16 = mybir.dt.bfloat16
    f32 = mybir.dt.float32

    nc = bacc.Bacc("TRN2", target_bir_lowering=False, debug=False)
    xT = nc.dram_tensor("xT", [P, D // P * C], bf16, kind="ExternalInput").ap()
    w1Y = nc.dram_tensor("w1Y", [P, H // P, D // P, P], bf16, kind="ExternalInput").ap()
    w1X = nc.dram_tensor("w1X", [P, H // P, D // P, P], bf16, kind="ExternalInput").ap()
    w2Y = nc.dram_tensor("w2Y", [P, D // P, H // P, P], bf16, kind="ExternalInput").ap()
    w2X = nc.dram_tensor("w2X", [P, D // P, H // P, P], bf16, kind="ExternalInput").ap()
    b1Y = nc.dram_tensor("b1Y", [P, H // P], f32, kind="ExternalInput").ap()
    b1X = nc.dram_tensor("b1X", [P, H // P], f32, kind="ExternalInput").ap()
    oT = nc.dram_tensor("oT", [P, D // P, C], f32, kind="ExternalOutput").ap()
    w1d = [w1Y, w1X]
    w2d = [w2Y, w2X]

    with tile.TileContext(nc) as tc:
        with (
            tc.tile_pool(name="const", bufs=1) as cpool,
            tc.tile_pool(name="resident", bufs=1) as rpool,
            tc.tile_pool(name="warm", bufs=1) as warmpool,
            tc.tile_pool(name="w1h", bufs=1) as w1hpool,
            tc.tile_pool(name="w1y", bufs=4) as w1ypool,
            tc.tile_pool(name="w1x", bufs=4) as w1xpool,
            tc.tile_pool(name="w2y", bufs=3) as w2ypool,
            tc.tile_pool(name="w2x", bufs=3) as w2xpool,
            tc.tile_pool(name="ost", bufs=4) as opool,
            tc.tile_pool(name="ps", bufs=4, space="PSUM") as pspool,
            tc.tile_pool(name="wps", bufs=1, space="PSUM") as wpspool,
        ):
            # ---- prologue ----
            # Few, large DMAs: small transfers pay ~2us completion latency
            # each and serialize their queue.  gpsimd streams x (one DMA
            # per block); sync streams w1Y; scalar takes biases + w1X.
            wsrc = warmpool.tile([P, WN], bf16)
            nc.vector.memset(wsrc[:], 0.0)

            # x on sync (HWDGE starts ~1us earlier than gpsimd's SWDGE)
            x_blks = []
            for blk, (st, sz) in enumerate(zip(starts, blocks)):
                xb = rpool.tile([P, D // P * sz], bf16, tag=f"xb{blk}")
                base = D // P * st
                nc.sync.dma_start(xb[:], xT[:, base : base + D // P * sz])
                x_blks.append(xb)

            w1_tiles = [{}, {}]  # stream -> {ho: tile}
            ty0 = w1hpool.tile([P, D // P, P], bf16, tag="w1y_h0")
            nc.gpsimd.dma_start(ty0[:], w1Y[:, 0])
            w1_tiles[0][0] = ty0
            ty1 = w1hpool.tile([P, D // P, P], bf16, tag="w1y_h1")
            nc.gpsimd.dma_start(ty1[:], w1Y[:, 1])
            w1_tiles[0][1] = ty1

            b1y_sb = cpool.tile([P, H // P], f32, tag="b1_0")
            b1x_sb = cpool.tile([P, H // P], f32, tag="b1_1")
            b1sb = [b1y_sb, b1x_sb]
            nc.scalar.dma_start(b1sb[0][:], b1Y[:])
            nc.scalar.dma_start(b1sb[1][:], b1X[:])
            tx = w1hpool.tile([P, D // P, P], bf16, tag="w1x_h0")
            nc.scalar.dma_start(tx[:], w1X[:, 0])
            w1_tiles[1][0] = tx

            # ---- PE warm-up: ends ~when first inputs land ----
            wps = wpspool.tile([P, WN], f32)
            for _ in range(NWARM):
                nc.tensor.matmul(wps[:], wsrc[:, :P], wsrc[:], start=True, stop=True)

            # ---- layer 1 ----
            h_sb = rpool.tile([P, H // P, C], bf16)
            w1pools = [w1ypool, w1xpool]
            w1eng = [nc.gpsimd, nc.scalar]

            def get_w1(s, ho):
                if ho not in w1_tiles[s]:
                    t = w1pools[s].tile([P, D // P, P], bf16, tag=f"w1s{s}")
                    w1eng[s].dma_start(t[:], w1d[s][:, ho])
                    w1_tiles[s][ho] = t
                return w1_tiles[s][ho]

            # lead order tracks DMA arrival: xb0 lands first (b0 rows run
            # while xb1 streams), then b1 rows, then the X block
            pairs = []
            if nb == 3 and LEAD >= 3:
                pairs = [(0, 0), (1, 0), (0, 1), (1, 1), (2, 0), (2, 1),
                         (0, 2), (1, 2), (2, 2)]
                rest = range(3, H // P)
            else:
                for ho in range(LEAD):
                    pairs += [(ho, b) for b in range(nY)]
                for ho in range(LEAD):
                    pairs.append((ho, nY))
                rest = range(LEAD, H // P)
            for ho in rest:
                pairs += [(ho, b) for b in range(nb)]

            for ho, blk in pairs:
                s = streams[blk]
                w1s = get_w1(s, ho)
                st, sz = starts[blk], blocks[blk]
                ps = pspool.tile([P, 512], f32, tag="ps")
                for ko in range(D // P):
                    nc.tensor.matmul(
                        ps[:, :sz],
                        w1s[:, ko, :],
                        x_blks[blk][:, ko * sz : (ko + 1) * sz],
                        start=(ko == 0),
                        stop=(ko == D // P - 1),
                    )
                nc.scalar.activation(
                    h_sb[:, ho, st : st + sz],
                    ps[:, :sz],
                    mybir.ActivationFunctionType.Gelu,
                    bias=b1sb[s][:, ho : ho + 1],
                )

            # ---- layer 2 ----
            w2pools = [w2ypool, w2xpool]
            w2eng = [nc.sync, nc.scalar]  # sync is free once x has landed
            for do in range(D // P):
                w2_t = [None, None]

                def get_w2(s, do=do):
                    if w2_t[s] is None:
                        t = w2pools[s].tile([P, H // P, P], bf16, tag=f"w2s{s}")
                        w2eng[s].dma_start(t[:], w2d[s][:, do])
                        w2_t[s] = t
                    return w2_t[s]

                order = list(range(nb))
                if do == D // P - 1:
                    order = order[::-1]  # finish on the smallest block
                for bi, blk in enumerate(order):
                    s = streams[blk]
                    w2s = get_w2(s)
                    st, sz = starts[blk], blocks[blk]
                    ps = pspool.tile([P, 512], f32, tag="ps")
                    for ko in range(H // P):
                        nc.tensor.matmul(
                            ps[:, :sz],
                            w2s[:, ko, :],
                            h_sb[:, ko, st : st + sz],
                            start=(ko == 0),
                            stop=(ko == H // P - 1),
                        )
                    last = do == D // P - 1 and bi == nb - 1
                    if not last:
                        ob = opool.tile([P, 512], f32, tag="ob")
                        nc.vector.tensor_copy(ob[:, :sz], ps[:, :sz])
                        nc.gpsimd.dma_start(oT[:, do, st : st + sz], ob[:, :sz])
                    else:
                        # final eviction on the critical path: split halves
                        # across two idle queues
                        hsz = sz // 2
                        ob = opool.tile([P, 512], f32, tag="ob")
                        nc.vector.tensor_copy(ob[:, :hsz], ps[:, :hsz])
                        nc.sync.dma_start(oT[:, do, st : st + hsz], ob[:, :hsz])
                        nc.vector.tensor_copy(ob[:, hsz:sz], ps[:, hsz:sz])
                        nc.scalar.dma_start(
                            oT[:, do, st + hsz : st + sz], ob[:, hsz:sz]
                        )

    nc.compile()
    return nc


def _route_host(x_flat, router_w):
    """Float64 router: returns per-expert (token_idx, combine_weight)."""
    logits = x_flat.astype(np.float64) @ router_w.astype(np.float64).T
    m = logits.max(axis=-1, keepdims=True)
    p = np.exp(logits - m)
    p /= p.sum(axis=-1, keepdims=True)
    order = np.argsort(-p, axis=-1)
    topi = order[:, :TOP_K]
    topw = np.take_along_axis(p, topi, axis=-1)
    topw /= topw.sum(axis=-1, keepdims=True)

    idx_list, wgt_list = [], []
    for e in range(E):
        mask = topi == e  # [T, TOP_K]; at most one True per row
        rows = np.nonzero(mask.any(axis=-1))[0]
        w = topw[rows][mask[rows]]
        idx_list.append(rows)
        wgt_list.append(w.astype(np.float32))
    return idx_list, wgt_list


def kernel(x, router_w, w1, b1, w2, b2):
    from concourse import bass_utils

    x = np.asarray(x)
    router_w = np.asarray(router_w)
    w1 = np.asarray(w1)
    b1 = np.asarray(b1)
    w2 = np.asarray(w2)
    b2 = np.asarray(b2)

    B, S, _ = x.shape
    T = B * S
    x_flat = x.reshape(T, D)

    idx_list, wgt_list = _route_host(x_flat, router_w)
    cnt = [len(i) for i in idx_list]
    SY, asn = _solve_layout(cnt)
    ybs = _split_y(SY)
    C = SY + SX

    # units: per expert, y_e Y-units then x_e X-units, consuming its tokens
    # in order; leftover units get expert 0 with zero tokens.
    yunits, xunits = [], []  # (expert, tok_slice)
    for e in range(E):
        xn, yn = asn[e]
        pos = 0
        for _ in range(yn):
            tk = idx_list[e][pos : pos + SY]
            yunits.append((e, pos, pos + len(tk)))
            pos += len(tk)
        for _ in range(xn):
            tk = idx_list[e][pos : pos + SX]
            xunits.append((e, pos, pos + len(tk)))
            pos += len(tk)
        assert pos >= cnt[e]
    while len(yunits) < 8:
        yunits.append((0, 0, 0))
    while len(xunits) < 8:
        xunits.append((0, 0, 0))

    key = (tuple(ybs), SX)
    if key not in _COMPILED:
        _COMPILED[key] = _build_ffn_kernel(*key)
    nc = _COMPILED[key]

    blocks = ybs + [SX]
    starts = [sum(blocks[:i]) for i in range(len(blocks))]
    bf = ml_dtypes.bfloat16

    def pack_w1(e):
        return np.ascontiguousarray(
            w1[e].reshape(H // P, P, D // P, P).transpose(3, 0, 2, 1)
        ).astype(bf)

    def pack_w2(e):
        return np.ascontiguousarray(
            w2[e].reshape(D // P, P, H // P, P).transpose(3, 0, 2, 1)
        ).astype(bf)

    def pack_b1(e):
        return np.ascontiguousarray(b1[e].reshape(H // P, P).T).astype(np.float32)

    w1p, w2p, b1p = {}, {}, {}
    for e in set(u[0] for u in yunits + xunits):
        w1p[e] = pack_w1(e)
        w2p[e] = pack_w2(e)
        b1p[e] = pack_b1(e)

    in_maps = []
    for core in range(E):
        eY, ylo, yhi = yunits[core]
        eX, xlo, xhi = xunits[core]
        xg = np.zeros((C, D), np.float32)
        xg[: yhi - ylo] = x_flat[idx_list[eY][ylo:yhi]]
        xg[SY : SY + xhi - xlo] = x_flat[idx_list[eX][xlo:xhi]]
        xT_full = xg.T.reshape(D // P, P, C).transpose(1, 0, 2)  # [128, 8, C]
        xT_d = np.concatenate(
            [
                xT_full[:, :, st : st + sz].reshape(P, -1)
                for st, sz in zip(starts, blocks)
            ],
            axis=1,
        ).astype(bf)
        in_maps.append(
            {
                "xT": xT_d,
                "w1Y": w1p[eY],
                "w1X": w1p[eX],
                "w2Y": w2p[eY],
                "w2X": w2p[eX],
                "b1Y": b1p[eY],
                "b1X": b1p[eX],
            }
        )

    res = bass_utils.run_bass_kernel_spmd(nc, in_maps, core_ids=list(range(E)))

    out = np.zeros((T, D), np.float32)
    for core in range(E):
        oTc = res.results[core]["oT"]  # [128, 8, C]
        o_g = oTc.transpose(1, 0, 2).reshape(D, C).T  # [C, D]
        eY, ylo, yhi = yunits[core]
        if yhi > ylo:
            idx = idx_list[eY][ylo:yhi]
            out[idx] += wgt_list[eY][ylo:yhi, None] * (
                o_g[: yhi - ylo] + b2[eY][None, :]
            )
        eX, xlo, xhi = xunits[core]
        if xhi > xlo:
            idx = idx_list[eX][xlo:xhi]
            out[idx] += wgt_list[eX][xlo:xhi, None] * (
                o_g[SY : SY + xhi - xlo] + b2[eX][None, :]
            )
    return out.reshape(B, S, D).astype(np.float32)


# revision 23
# speedup vs baseline: 1.0066x; 1.0066x over previous
"""MoE top-2 routing kernel for 8 Trainium2 NeuronCores.

Strategy (expert-parallel with 2-stream load balancing):
  - Host computes the (tiny) router in float64: logits -> softmax -> top-2 ->
    renormalize.  Selection was verified tie-safe: min prob gap between
    2nd/3rd expert is ~8e-6 while cross-backend fp32 logit noise is ~3e-7.
  - The 8192 (token, expert) pairs are packed into 16 capacity units:
    8 "X" units of SX=512 tokens and 8 "Y" units of SY tokens, each unit
    single-expert.  A small solver picks the minimal SY such that every
    expert's token count is covered by an integral number of units
    (experts can span units on different cores).  Each core gets one X
    unit and one Y unit -> per-core capacity C = SX + SY (1048 for the
    staged routing, vs 1088 for plain expert-per-core), with two weight
    streams per core instead of one.
  - Each core runs a dense FFN over its C tokens in nb blocks (Y split
    into <=512-token blocks + the X block), with D/H features on the
    partition axis end-to-end, so no on-device transposes are needed.
        hT = gelu(w1T.T-contractions)   (PSUM fp32 accum, bias fused in ACT)
        oT = w2-contractions over hT
  - Startup is DMA-limited (~100 GB/s per HWDGE queue): the first-needed
    transfers (x block 0, w1Y tiles 0-1) are split across 4 engine queues
    and issued before anything else; PE warm-up matmuls are sized to end
    right when the first real inputs land so HAM stays at 8/8.
  - Host applies combine weights and scatter-adds per-unit outputs back
    into the full [B,S,D] output.

Per-core layouts (D=1024, H=4096, C tokens, P=128):
  xT  [128, 8*C]         bf16   block-major: xT[p, ko*sz+t] = x_blk[t, ko*128+p]
  w1Y/w1X [128, 32, 8, 128] bf16  w1t[p, ho, ko, j] = w1[e][ho*128+j, ko*128+p]
  w2Y/w2X [128, 8, 32, 128] bf16  w2t[p, do, ko, j] = w2[e][do*128+j, ko*128+p]
  b1Y/b1X [128, 32]      f32    b1t[p, ho]         = b1[e][ho*128+p]
  oT  [128, 8, C]        f32    oT[p, do, t]       = o_blk[t, do*128+p]
"""

import numpy as np
import ml_dtypes

TOP_K = 2
P = 128
D = 1024
H = 4096
E = 8
SX = 512
NWARM = 26
WN = 256
LEAD = 3

_COMPILED = {}  # (ybs, sx) -> compiled Bacc instance


def _split_y(SY):
    """Split SY into <=512 blocks (multiples of 8 assumed), smallest first."""
    n = -(-SY // 512)
    base = ((SY // n + 7) // 8) * 8
    first = SY - base * (n - 1)
    assert 0 < first <= base <= 512, (SY, n, base, first)
    return [first] + [base] * (n - 1)


def _try_assign(cnt, SY):
    """Cover counts with 8 X-units (SX) and 8 Y-units (SY); returns
    [(x_e, y_e)] per expert or None."""
    order = sorted(range(E), key=lambda e: -cnt[e])
    res = [None] * E

    def opts(need, xb, yb):
        out = []
        for y in range(min(yb, 8) + 1):
            rem = need - y * SY
            x = max(0, -(-rem // SX)) if rem > 0 else 0
            if x <= xb:
                tot = x * SX + y * SY
                if tot >= need:
                    out.append((tot - need, x, y))
        out.sort()
        # drop dominated (same or more of both units)
        seen = []
        for w, x, y in out:
            if not any(x >= sx and y >= sy for sx, sy in seen):
                seen.append((x, y))
        return seen

    def dfs(i, xb, yb):
        if i == E:
            return True
        e = order[i]
        for x, y in opts(cnt[e], xb, yb):
            res[e] = (x, y)
            if dfs(i + 1, xb - x, yb - y):
                return True
        res[e] = None
        return False

    return res if dfs(0, 8, 8) else None


def _solve_layout(cnt):
    for SY in range(256, 8 * 4096, 8):
        asn = _try_assign(cnt, SY)
        if asn is not None:
            return SY, asn
    raise RuntimeError("no unit assignment found")


def _build_ffn_kernel(ybs, sx):
    import concourse.mybir as mybir
    import concourse.tile as tile
    from concourse import bacc

    ybs = list(ybs)
    nY = len(ybs)
    blocks = ybs + [sx]          # sizes, processing order
    streams = [0] * nY + [1]     # 0 = Y weights, 1 = X weights
    starts = [sum(blocks[:i]) for i in range(len(blocks))]
    nb = len(blocks)
    C = sum(blocks)
    bf16 = mybir.dt.bfloat16
    f32 = mybir.dt.float32

    nc = bacc.Bacc("TRN2", target_bir_lowering=False, debug=False)
    xT = nc.dram_tensor("xT", [P, D // P * C], bf16, kind="ExternalInput").ap()
    w1Y = nc.dram_tensor("w1Y", [P, H // P, D // P, P], bf16, kind="ExternalInput").ap()
    w1X = nc.dram_tensor("w1X", [P, H // P, D // P, P], bf16, kind="ExternalInput").ap()
    w2Y = nc.dram_tensor("w2Y", [P, D // P, H // P, P], bf16, kind="ExternalInput").ap()
    w2X = nc.dram_tensor("w2X", [P, D // P, H // P, P], bf16, kind="ExternalInput").ap()
    b1Y = nc.dram_tensor("b1Y", [P, H // P], f32, kind="ExternalInput").ap()
    b1X = nc.dram_tensor("b1X", [P, H // P], f32, kind="ExternalInput").ap()
    oT = nc.dram_tensor("oT", [P, D // P, C], f32, kind="ExternalOutput").ap()
    w1d = [w1Y, w1X]
    w2d = [w2Y, w2X]

    with tile.TileContext(nc) as tc:
        with (
            tc.tile_pool(name="const", bufs=1) as cpool,
            tc.tile_pool(name="resident", bufs=1) as rpool,
            tc.tile_pool(name="warm", bufs=1) as warmpool,
            tc.tile_pool(name="w1h", bufs=1) as w1hpool,
            tc.tile_pool(name="w1y", bufs=4) as w1ypool,
            tc.tile_pool(name="w1x", bufs=4) as w1xpool,
            tc.tile_pool(name="w2y", bufs=3) as w2ypool,
            tc.tile_pool(name="w2x", bufs=3) as w2xpool,
            tc.tile_pool(name="ost", bufs=4) as opool,
            tc.tile_pool(name="ps", bufs=4, space="PSUM") as pspool,
            tc.tile_pool(name="wps", bufs=1, space="PSUM") as wpspool,
        ):
            # ---- prologue ----
            # Few, large DMAs: small transfers pay ~2us completion latency
            # each and serialize their queue.  gpsimd streams x (one DMA
            # per block); sync streams w1Y; scalar takes biases + w1X.
            wsrc = warmpool.tile([P, WN], bf16)
            nc.vector.memset(wsrc[:], 0.0)

            # x on sync (HWDGE starts ~1us earlier than gpsimd's SWDGE)
            x_blks = []
            for blk, (st, sz) in enumerate(zip(starts, blocks)):
                xb = rpool.tile([P, D // P * sz], bf16, tag=f"xb{blk}")
                base = D // P * st
                nc.sync.dma_start(xb[:], xT[:, base : base + D // P * sz])
                x_blks.append(xb)

            w1_tiles = [{}, {}]  # stream -> {ho: tile}
            ty0 = w1hpool.tile([P, D // P, P], bf16, tag="w1y_h0")
            nc.gpsimd.dma_start(ty0[:], w1Y[:, 0])
            w1_tiles[0][0] = ty0
            ty1 = w1hpool.tile([P, D // P, P], bf16, tag="w1y_h1")
            nc.gpsimd.dma_start(ty1[:], w1Y[:, 1])
            w1_tiles[0][1] = ty1

            b1y_sb = cpool.tile([P, H // P], f32, tag="b1_0")
            b1x_sb = cpool.tile([P, H // P], f32, tag="b1_1")
            b1sb = [b1y_sb, b1x_sb]
            nc.scalar.dma_start(b1sb[0][:], b1Y[:])
            nc.scalar.dma_start(b1sb[1][:], b1X[:])
            # w1X tiles 0-1 hoisted on scalar ahead of any ACT; the rest of
            # the w1X stream rides sync (free after x) so its doorbells
            # never queue behind ACT completions in the scalar FIFO
            tx = w1hpool.tile([P, D // P, P], bf16, tag="w1x_h0")
            nc.scalar.dma_start(tx[:], w1X[:, 0])
            w1_tiles[1][0] = tx
            tx1 = w1hpool.tile([P, D // P, P], bf16, tag="w1x_h1")
            nc.scalar.dma_start(tx1[:], w1X[:, 1])
            w1_tiles[1][1] = tx1

            # ---- PE warm-up: ends ~when first inputs land ----
            wps = wpspool.tile([P, WN], f32)
            for _ in range(NWARM):
                nc.tensor.matmul(wps[:], wsrc[:, :P], wsrc[:], start=True, stop=True)

            # ---- layer 1 ----
            h_sb = rpool.tile([P, H // P, C], bf16)
            w1pools = [w1ypool, w1xpool]
            w1eng = [nc.gpsimd, nc.sync]

            def get_w1(s, ho):
                if ho not in w1_tiles[s]:
                    t = w1pools[s].tile([P, D // P, P], bf16, tag=f"w1s{s}")
                    w1eng[s].dma_start(t[:], w1d[s][:, ho])
                    w1_tiles[s][ho] = t
                return w1_tiles[s][ho]

            # lead order tracks DMA arrival: xb0 lands first (b0 rows run
            # while xb1 streams), then b1 rows, then the X block
            pairs = []
            if nb == 3 and LEAD >= 3:
                pairs = [(0, 0), (1, 0), (0, 1), (1, 1), (2, 0), (2, 1),
                         (0, 2), (1, 2), (2, 2)]
                rest = range(3, H // P)
            else:
                for ho in range(LEAD):
                    pairs += [(ho, b) for b in range(nY)]
                for ho in range(LEAD):
                    pairs.append((ho, nY))
                rest = range(LEAD, H // P)
            for ho in rest:
                pairs += [(ho, b) for b in range(nb)]

            for ho, blk in pairs:
                s = streams[blk]
                w1s = get_w1(s, ho)
                st, sz = starts[blk], blocks[blk]
                ps = pspool.tile([P, 512], f32, tag="ps")
                for ko in range(D // P):
                    nc.tensor.matmul(
                        ps[:, :sz],
                        w1s[:, ko, :],
                        x_blks[blk][:, ko * sz : (ko + 1) * sz],
                        start=(ko == 0),
                        stop=(ko == D // P - 1),
                    )
                nc.scalar.activation(
                    h_sb[:, ho, st : st + sz],
                    ps[:, :sz],
                    mybir.ActivationFunctionType.Gelu,
                    bias=b1sb[s][:, ho : ho + 1],
                )

            # ---- layer 2 ----
            w2pools = [w2ypool, w2xpool]
            w2eng = [nc.sync, nc.scalar]  # sync is free once x has landed
            for do in range(D // P):
                w2_t = [None, None]

                def get_w2(s, do=do):
                    if w2_t[s] is None:
                        t = w2pools[s].tile([P, H // P, P], bf16, tag=f"w2s{s}")
                        w2eng[s].dma_start(t[:], w2d[s][:, do])
                        w2_t[s] = t
                    return w2_t[s]

                order = list(range(nb))
                if do == D // P - 1:
                    order = order[::-1]  # finish on the smallest block
                for bi, blk in enumerate(order):
                    s = streams[blk]
                    w2s = get_w2(s)
                    st, sz = starts[blk], blocks[blk]
                    ps = pspool.tile([P, 512], f32, tag="ps")
                    for ko in range(H // P):
                        nc.tensor.matmul(
                            ps[:, :sz],
                            w2s[:, ko, :],
                            h_sb[:, ko, st : st + sz],
                            start=(ko == 0),
                            stop=(ko == H // P - 1),
                        )
                    last = do == D // P - 1 and bi == nb - 1
                    if not last:
                        ob = opool.tile([P, 512], f32, tag="ob")
                        nc.vector.tensor_copy(ob[:, :sz], ps[:, :sz])
                        nc.gpsimd.dma_start(oT[:, do, st : st + sz], ob[:, :sz])
                    else:
                        # final eviction on the critical path: split halves
                        # across two idle queues
                        hsz = sz // 2
                        ob = opool.tile([P, 512], f32, tag="ob")
                        nc.vector.tensor_copy(ob[:, :hsz], ps[:, :hsz])
                        nc.sync.dma_start(oT[:, do, st : st + hsz], ob[:, :hsz])
                        nc.vector.tensor_copy(ob[:, hsz:sz], ps[:, hsz:sz])
                        nc.scalar.dma_start(
                            oT[:, do, st + hsz : st + sz], ob[:, hsz:sz]
                        )

    nc.compile()
    return nc


def _route_host(x_flat, router_w):
    """Float64 router: returns per-expert (token_idx, combine_weight)."""
    logits = x_flat.astype(np.float64) @ router_w.astype(np.float64).T
    m = logits.max(axis=-1, keepdims=True)
    p = np.exp(logits - m)
    p /= p.sum(axis=-1, keepdims=True)
    order = np.argsort(-p, axis=-1)
    topi = order[:, :TOP_K]
    topw = np.take_along_axis(p, topi, axis=-1)
    topw /= topw.sum(axis=-1, keepdims=True)

    idx_list, wgt_list = [], []
    for e in range(E):
        mask = topi == e  # [T, TOP_K]; at most one True per row
        rows = np.nonzero(mask.any(axis=-1))[0]
        w = topw[rows][mask[rows]]
        idx_list.append(rows)
        wgt_list.append(w.astype(np.float32))
    return idx_list, wgt_list


def kernel(x, router_w, w1, b1, w2, b2):
    from concourse import bass_utils

    x = np.asarray(x)
    router_w = np.asarray(router_w)
    w1 = np.asarray(w1)
    b1 = np.asarray(b1)
    w2 = np.asarray(w2)
    b2 = np.asarray(b2)

    B, S, _ = x.shape
    T = B * S
    x_flat = x.reshape(T, D)

    idx_list, wgt_list = _route_host(x_flat, router_w)
    cnt = [len(i) for i in idx_list]
    SY, asn = _solve_layout(cnt)
    ybs = _split_y(SY)
    C = SY + SX

    # units: per expert, y_e Y-units then x_e X-units, consuming its tokens
    # in order; leftover units get expert 0 with zero tokens.
    yunits, xunits = [], []  # (expert, tok_slice)
    for e in range(E):
        xn, yn = asn[e]
        pos = 0
        for _ in range(yn):
            tk = idx_list[e][pos : pos + SY]
            yunits.append((e, pos, pos + len(tk)))
            pos += len(tk)
        for _ in range(xn):
            tk = idx_list[e][pos : pos + SX]
            xunits.append((e, pos, pos + len(tk)))
            pos += len(tk)
        assert pos >= cnt[e]
    while len(yunits) < 8:
        yunits.append((0, 0, 0))
    while len(xunits) < 8:
        xunits.append((0, 0, 0))

    key = (tuple(ybs), SX)
    if key not in _COMPILED:
        _COMPILED[key] = _build_ffn_kernel(*key)
    nc = _COMPILED[key]

    blocks = ybs + [SX]
    starts = [sum(blocks[:i]) for i in range(len(blocks))]
    bf = ml_dtypes.bfloat16

    def pack_w1(e):
        return np.ascontiguousarray(
            w1[e].reshape(H // P, P, D // P, P).transpose(3, 0, 2, 1)
        ).astype(bf)

    def pack_w2(e):
        return np.ascontiguousarray(
            w2[e].reshape(D // P, P, H // P, P).transpose(3, 0, 2, 1)
        ).astype(bf)

    def pack_b1(e):
        return np.ascontiguousarray(b1[e].reshape(H // P, P).T).astype(np.float32)

    w1p, w2p, b1p = {}, {}, {}
    for e in set(u[0] for u in yunits + xunits):
        w1p[e] = pack_w1(e)
        w2p[e] = pack_w2(e)
        b1p[e] = pack_b1(e)

    in_maps = []
    for core in range(E):
        eY, ylo, yhi = yunits[core]
        eX, xlo, xhi = xunits[core]
        xg = np.zeros((C, D), np.float32)
        xg[: yhi - ylo] = x_flat[idx_list[eY][ylo:yhi]]
        xg[SY : SY + xhi - xlo] = x_flat[idx_list[eX][xlo:xhi]]
        xT_full = xg.T.reshape(D // P, P, C).transpose(1, 0, 2)  # [128, 8, C]
        xT_d = np.concatenate(
            [
                xT_full[:, :, st : st + sz].reshape(P, -1)
                for st, sz in zip(starts, blocks)
            ],
            axis=1,
        ).astype(bf)
        in_maps.append(
            {
                "xT": xT_d,
                "w1Y": w1p[eY],
                "w1X": w1p[eX],
                "w2Y": w2p[eY],
                "w2X": w2p[eX],
                "b1Y": b1p[eY],
                "b1X": b1p[eX],
            }
        )

    res = bass_utils.run_bass_kernel_spmd(nc, in_maps, core_ids=list(range(E)))

    out = np.zeros((T, D), np.float32)
    for core in range(E):
        oTc = res.results[core]["oT"]  # [128, 8, C]
        o_g = oTc.transpose(1, 0, 2).reshape(D, C).T  # [C, D]
        eY, ylo, yhi = yunits[core]
        if yhi > ylo:
            idx = idx_list[eY][ylo:yhi]
            out[idx] += wgt_list[eY][ylo:yhi, None] * (
                o_g[: yhi - ylo] + b2[eY][None, :]
            )
        eX, xlo, xhi = xunits[core]
        if xhi > xlo:
            idx = idx_list[eX][xlo:xhi]
            out[idx] += wgt_list[eX][xlo:xhi, None] * (
                o_g[SY : SY + xhi - xlo] + b2[eX][None, :]
            )
    return out.reshape(B, S, D).astype(np.float32)


# revision 24
# speedup vs baseline: 1.0128x; 1.0063x over previous
"""MoE top-2 routing kernel for 8 Trainium2 NeuronCores.

Strategy (expert-parallel with 2-stream load balancing):
  - Host computes the (tiny) router in float64: logits -> softmax -> top-2 ->
    renormalize.  Selection was verified tie-safe: min prob gap between
    2nd/3rd expert is ~8e-6 while cross-backend fp32 logit noise is ~3e-7.
  - The 8192 (token, expert) pairs are packed into 16 capacity units:
    8 "X" units of SX=512 tokens and 8 "Y" units of SY tokens, each unit
    single-expert.  A small solver picks the minimal SY such that every
    expert's token count is covered by an integral number of units
    (experts can span units on different cores).  Each core gets one X
    unit and one Y unit -> per-core capacity C = SX + SY (1048 for the
    staged routing, vs 1088 for plain expert-per-core), with two weight
    streams per core instead of one.
  - Each core runs a dense FFN over its C tokens in nb blocks (Y split
    into <=512-token blocks + the X block), with D/H features on the
    partition axis end-to-end, so no on-device transposes are needed.
        hT = gelu(w1T.T-contractions)   (PSUM fp32 accum, bias fused in ACT)
        oT = w2-contractions over hT
  - Startup is DMA-limited (~100 GB/s per HWDGE queue): the first-needed
    transfers (x block 0, w1Y tiles 0-1) are split across 4 engine queues
    and issued before anything else; PE warm-up matmuls are sized to end
    right when the first real inputs land so HAM stays at 8/8.
  - Host applies combine weights and scatter-adds per-unit outputs back
    into the full [B,S,D] output.

Per-core layouts (D=1024, H=4096, C tokens, P=128):
  xT  [128, 8*C]         bf16   block-major: xT[p, ko*sz+t] = x_blk[t, ko*128+p]
  w1Y/w1X [128, 32, 8, 128] bf16  w1t[p, ho, ko, j] = w1[e][ho*128+j, ko*128+p]
  w2Y/w2X [128, 8, 32, 128] bf16  w2t[p, do, ko, j] = w2[e][do*128+j, ko*128+p]
  b1Y/b1X [128, 32]      f32    b1t[p, ho]         = b1[e][ho*128+p]
  oT  [128, 8, C]        f32    oT[p, do, t]       = o_blk[t, do*128+p]
"""

import numpy as np
import ml_dtypes

TOP_K = 2
P = 128
D = 1024
H = 4096
E = 8
SX = 512
NWARM = 16
WN = 256
LEAD = 3

_COMPILED = {}  # (ybs, sx) -> compiled Bacc instance


def _split_y(SY):
    """Split SY into <=512 blocks (multiples of 8 assumed), smallest first."""
    n = -(-SY // 512)
    base = ((SY // n + 7) // 8) * 8
    first = SY - base * (n - 1)
    assert 0 < first <= base <= 512, (SY, n, base, first)
    return [first] + [base] * (n - 1)


def _try_assign(cnt, SY):
    """Cover counts with 8 X-units (SX) and 8 Y-units (SY); returns
    [(x_e, y_e)] per expert or None."""
    order = sorted(range(E), key=lambda e: -cnt[e])
    res = [None] * E

    def opts(need, xb, yb):
        out = []
        for y in range(min(yb, 8) + 1):
            rem = need - y * SY
            x = max(0, -(-rem // SX)) if rem > 0 else 0
            if x <= xb:
                tot = x * SX + y * SY
                if tot >= need:
                    out.append((tot - need, x, y))
        out.sort()
        # drop dominated (same or more of both units)
        seen = []
        for w, x, y in out:
            if not any(x >= sx and y >= sy for sx, sy in seen):
                seen.append((x, y))
        return seen

    def dfs(i, xb, yb):
        if i == E:
            return True
        e = order[i]
        for x, y in opts(cnt[e], xb, yb):
            res[e] = (x, y)
            if dfs(i + 1, xb - x, yb - y):
                return True
        res[e] = None
        return False

    return res if dfs(0, 8, 8) else None


def _solve_layout(cnt):
    for SY in range(256, 8 * 4096, 8):
        asn = _try_assign(cnt, SY)
        if asn is not None:
            return SY, asn
    raise RuntimeError("no unit assignment found")


def _build_ffn_kernel(ybs, sx):
    import concourse.mybir as mybir
    import concourse.tile as tile
    from concourse import bacc

    ybs = list(ybs)
    nY = len(ybs)
    blocks = ybs + [sx]          # sizes, processing order
    streams = [0] * nY + [1]     # 0 = Y weights, 1 = X weights
    starts = [sum(blocks[:i]) for i in range(len(blocks))]
    nb = len(blocks)
    C = sum(blocks)
    bf16 = mybir.dt.bfloat16
    f32 = mybir.dt.float32

    nc = bacc.Bacc("TRN2", target_bir_lowering=False, debug=False)
    xT = nc.dram_tensor("xT", [P, D // P * C], bf16, kind="ExternalInput").ap()
    w1Y = nc.dram_tensor("w1Y", [P, H // P, D // P, P], bf16, kind="ExternalInput").ap()
    w1X = nc.dram_tensor("w1X", [P, H // P, D // P, P], bf16, kind="ExternalInput").ap()
    w2Y = nc.dram_tensor("w2Y", [P, D // P, H // P, P], bf16, kind="ExternalInput").ap()
    w2X = nc.dram_tensor("w2X", [P, D // P, H // P, P], bf16, kind="ExternalInput").ap()
    b1Y = nc.dram_tensor("b1Y", [P, H // P], f32, kind="ExternalInput").ap()
    b1X = nc.dram_tensor("b1X", [P, H // P], f32, kind="ExternalInput").ap()
    oT = nc.dram_tensor("oT", [P, D // P, C], f32, kind="ExternalOutput").ap()
    w1d = [w1Y, w1X]
    w2d = [w2Y, w2X]

    with tile.TileContext(nc) as tc:
        with (
            tc.tile_pool(name="const", bufs=1) as cpool,
            tc.tile_pool(name="resident", bufs=1) as rpool,
            tc.tile_pool(name="warm", bufs=1) as warmpool,
            tc.tile_pool(name="w1h", bufs=1) as w1hpool,
            tc.tile_pool(name="w1y", bufs=4) as w1ypool,
            tc.tile_pool(name="w1x", bufs=4) as w1xpool,
            tc.tile_pool(name="w2y", bufs=3) as w2ypool,
            tc.tile_pool(name="w2x", bufs=3) as w2xpool,
            tc.tile_pool(name="ost", bufs=4) as opool,
            tc.tile_pool(name="ps", bufs=4, space="PSUM") as pspool,
            tc.tile_pool(name="wps", bufs=1, space="PSUM") as wpspool,
        ):
            # ---- prologue ----
            # Few, large DMAs: small transfers pay ~2us completion latency
            # each and serialize their queue.  gpsimd streams x (one DMA
            # per block); sync streams w1Y; scalar takes biases + w1X.
            wsrc = warmpool.tile([P, WN], bf16)
            nc.vector.memset(wsrc[:], 0.0)

            # x on sync (HWDGE starts ~1us earlier than gpsimd's SWDGE)
            x_blks = []
            for blk, (st, sz) in enumerate(zip(starts, blocks)):
                xb = rpool.tile([P, D // P * sz], bf16, tag=f"xb{blk}")
                base = D // P * st
                nc.sync.dma_start(xb[:], xT[:, base : base + D // P * sz])
                x_blks.append(xb)

            w1_tiles = [{}, {}]  # stream -> {ho: tile}
            ty0 = w1hpool.tile([P, D // P, P], bf16, tag="w1y_h0")
            nc.gpsimd.dma_start(ty0[:], w1Y[:, 0])
            w1_tiles[0][0] = ty0
            ty1 = w1hpool.tile([P, D // P, P], bf16, tag="w1y_h1")
            nc.gpsimd.dma_start(ty1[:], w1Y[:, 1])
            w1_tiles[0][1] = ty1

            b1y_sb = cpool.tile([P, H // P], f32, tag="b1_0")
            b1x_sb = cpool.tile([P, H // P], f32, tag="b1_1")
            b1sb = [b1y_sb, b1x_sb]
            nc.scalar.dma_start(b1sb[0][:], b1Y[:])
            nc.scalar.dma_start(b1sb[1][:], b1X[:])
            tx = w1hpool.tile([P, D // P, P], bf16, tag="w1x_h0")
            nc.scalar.dma_start(tx[:], w1X[:, 0])
            w1_tiles[1][0] = tx

            # ---- PE warm-up: ends ~when first inputs land ----
            wps = wpspool.tile([P, WN], f32)
            for _ in range(NWARM):
                nc.tensor.matmul(wps[:], wsrc[:, :P], wsrc[:], start=True, stop=True)

            # ---- layer 1 ----
            h_sb = rpool.tile([P, H // P, C], bf16)
            w1pools = [w1ypool, w1xpool]
            w1eng = [nc.gpsimd, nc.scalar]

            def get_w1(s, ho):
                if ho not in w1_tiles[s]:
                    t = w1pools[s].tile([P, D // P, P], bf16, tag=f"w1s{s}")
                    w1eng[s].dma_start(t[:], w1d[s][:, ho])
                    w1_tiles[s][ho] = t
                return w1_tiles[s][ho]

            # lead order tracks DMA arrival: xb0 lands first (b0 rows run
            # while xb1 streams), then b1 rows, then the X block
            pairs = []
            if nb == 3 and LEAD >= 3:
                pairs = [(0, 0), (1, 0), (0, 1), (1, 1), (2, 0), (2, 1),
                         (0, 2), (1, 2), (2, 2)]
                rest = range(3, H // P)
            else:
                for ho in range(LEAD):
                    pairs += [(ho, b) for b in range(nY)]
                for ho in range(LEAD):
                    pairs.append((ho, nY))
                rest = range(LEAD, H // P)
            for ho in rest:
                pairs += [(ho, b) for b in range(nb)]

            for ho, blk in pairs:
                s = streams[blk]
                w1s = get_w1(s, ho)
                st, sz = starts[blk], blocks[blk]
                ps = pspool.tile([P, 512], f32, tag="ps")
                for ko in range(D // P):
                    nc.tensor.matmul(
                        ps[:, :sz],
                        w1s[:, ko, :],
                        x_blks[blk][:, ko * sz : (ko + 1) * sz],
                        start=(ko == 0),
                        stop=(ko == D // P - 1),
                    )
                nc.scalar.activation(
                    h_sb[:, ho, st : st + sz],
                    ps[:, :sz],
                    mybir.ActivationFunctionType.Gelu,
                    bias=b1sb[s][:, ho : ho + 1],
                )

            # ---- layer 2 ----
            w2pools = [w2ypool, w2xpool]
            w2eng = [nc.sync, nc.scalar]  # sync is free once x has landed
            for do in range(D // P):
                w2_t = [None, None]

                def get_w2(s, do=do):
                    if w2_t[s] is None:
                        t = w2pools[s].tile([P, H // P, P], bf16, tag=f"w2s{s}")
                        w2eng[s].dma_start(t[:], w2d[s][:, do])
                        w2_t[s] = t
                    return w2_t[s]

                order = list(range(nb))
                if do == D // P - 1:
                    order = order[::-1]  # finish on the smallest block
                for bi, blk in enumerate(order):
                    s = streams[blk]
                    w2s = get_w2(s)
                    st, sz = starts[blk], blocks[blk]
                    ps = pspool.tile([P, 512], f32, tag="ps")
                    for ko in range(H // P):
                        nc.tensor.matmul(
                            ps[:, :sz],
                            w2s[:, ko, :],
                            h_sb[:, ko, st : st + sz],
                            start=(ko == 0),
                            stop=(ko == H // P - 1),
                        )
                    last = do == D // P - 1 and bi == nb - 1
                    if not last:
                        ob = opool.tile([P, 512], f32, tag="ob")
                        nc.vector.tensor_copy(ob[:, :sz], ps[:, :sz])
                        nc.gpsimd.dma_start(oT[:, do, st : st + sz], ob[:, :sz])
                    else:
                        # final eviction on the critical path: split halves
                        # across two idle queues
                        hsz = sz // 2
                        ob = opool.tile([P, 512], f32, tag="ob")
                        nc.vector.tensor_copy(ob[:, :hsz], ps[:, :hsz])
                        nc.sync.dma_start(oT[:, do, st : st + hsz], ob[:, :hsz])
                        nc.vector.tensor_copy(ob[:, hsz:sz], ps[:, hsz:sz])
                        nc.scalar.dma_start(
                            oT[:, do, st + hsz : st + sz], ob[:, hsz:sz]
                        )

    nc.compile()
    return nc


def _route_host(x_flat, router_w):
    """Float64 router: returns per-expert (token_idx, combine_weight)."""
    logits = x_flat.astype(np.float64) @ router_w.astype(np.float64).T
    m = logits.max(axis=-1, keepdims=True)
    p = np.exp(logits - m)
    p /= p.sum(axis=-1, keepdims=True)
    order = np.argsort(-p, axis=-1)
    topi = order[:, :TOP_K]
    topw = np.take_along_axis(p, topi, axis=-1)
    topw /= topw.sum(axis=-1, keepdims=True)

    idx_list, wgt_list = [], []
    for e in range(E):
        mask = topi == e  # [T, TOP_K]; at most one True per row
        rows = np.nonzero(mask.any(axis=-1))[0]
        w = topw[rows][mask[rows]]
        idx_list.append(rows)
        wgt_list.append(w.astype(np.float32))
    return idx_list, wgt_list


def kernel(x, router_w, w1, b1, w2, b2):
    from concourse import bass_utils

    x = np.asarray(x)
    router_w = np.asarray(router_w)
    w1 = np.asarray(w1)
    b1 = np.asarray(b1)
    w2 = np.asarray(w2)
    b2 = np.asarray(b2)

    B, S, _ = x.shape
    T = B * S
    x_flat = x.reshape(T, D)

    idx_list, wgt_list = _route_host(x_flat, router_w)
    cnt = [len(i) for i in idx_list]
    SY, asn = _solve_layout(cnt)
    ybs = _split_y(SY)
    C = SY + SX

    # units: per expert, y_e Y-units then x_e X-units, consuming its tokens
    # in order; leftover units get expert 0 with zero tokens.
    yunits, xunits = [], []  # (expert, tok_slice)
    for e in range(E):
        xn, yn = asn[e]
        pos = 0
        for _ in range(yn):
            tk = idx_list[e][pos : pos + SY]
            yunits.append((e, pos, pos + len(tk)))
            pos += len(tk)
        for _ in range(xn):
            tk = idx_list[e][pos : pos + SX]
            xunits.append((e, pos, pos + len(tk)))
            pos += len(tk)
        assert pos >= cnt[e]
    while len(yunits) < 8:
        yunits.append((0, 0, 0))
    while len(xunits) < 8:
        xunits.append((0, 0, 0))

    key = (tuple(ybs), SX)
    if key not in _COMPILED:
        _COMPILED[key] = _build_ffn_kernel(*key)
    nc = _COMPILED[key]

    blocks = ybs + [SX]
    starts = [sum(blocks[:i]) for i in range(len(blocks))]
    bf = ml_dtypes.bfloat16

    def pack_w1(e):
        return np.ascontiguousarray(
            w1[e].reshape(H // P, P, D // P, P).transpose(3, 0, 2, 1)
        ).astype(bf)

    def pack_w2(e):
        return np.ascontiguousarray(
            w2[e].reshape(D // P, P, H // P, P).transpose(3, 0, 2, 1)
        ).astype(bf)

    def pack_b1(e):
        return np.ascontiguousarray(b1[e].reshape(H // P, P).T).astype(np.float32)

    w1p, w2p, b1p = {}, {}, {}
    for e in set(u[0] for u in yunits + xunits):
        w1p[e] = pack_w1(e)
        w2p[e] = pack_w2(e)
        b1p[e] = pack_b1(e)

    in_maps = []
    for core in range(E):
        eY, ylo, yhi = yunits[core]
        eX, xlo, xhi = xunits[core]
        xg = np.zeros((C, D), np.float32)
        xg[: yhi - ylo] = x_flat[idx_list[eY][ylo:yhi]]
        xg[SY : SY + xhi - xlo] = x_flat[idx_list[eX][xlo:xhi]]
        xT_full = xg.T.reshape(D // P, P, C).transpose(1, 0, 2)  # [128, 8, C]
        xT_d = np.concatenate(
            [
                xT_full[:, :, st : st + sz].reshape(P, -1)
                for st, sz in zip(starts, blocks)
            ],
            axis=1,
        ).astype(bf)
        in_maps.append(
            {
                "xT": xT_d,
                "w1Y": w1p[eY],
                "w1X": w1p[eX],
                "w2Y": w2p[eY],
                "w2X": w2p[eX],
                "b1Y": b1p[eY],
                "b1X": b1p[eX],
            }
        )

    res = bass_utils.run_bass_kernel_spmd(nc, in_maps, core_ids=list(range(E)))

    out = np.zeros((T, D), np.float32)
    for core in range(E):
        oTc = res.results[core]["oT"]  # [128, 8, C]
        o_g = oTc.transpose(1, 0, 2).reshape(D, C).T  # [C, D]
        eY, ylo, yhi = yunits[core]
        if yhi > ylo:
            idx = idx_list[eY][ylo:yhi]
            out[idx] += wgt_list[eY][ylo:yhi, None] * (
                o_g[: yhi - ylo] + b2[eY][None, :]
            )
        eX, xlo, xhi = xunits[core]
        if xhi > xlo:
            idx = idx_list[eX][xlo:xhi]
            out[idx] += wgt_list[eX][xlo:xhi, None] * (
                o_g[SY : SY + xhi - xlo] + b2[eX][None, :]
            )
    return out.reshape(B, S, D).astype(np.float32)


# revision 28
# speedup vs baseline: 1.0150x; 1.0021x over previous
"""MoE top-2 routing kernel for 8 Trainium2 NeuronCores.

Strategy (expert-parallel with 2-stream load balancing):
  - Host computes the (tiny) router in float64: logits -> softmax -> top-2 ->
    renormalize.  Selection was verified tie-safe: min prob gap between
    2nd/3rd expert is ~8e-6 while cross-backend fp32 logit noise is ~3e-7.
  - The 8192 (token, expert) pairs are packed into 16 capacity units:
    8 "X" units of SX=512 tokens and 8 "Y" units of SY tokens, each unit
    single-expert.  A small solver picks the minimal SY such that every
    expert's token count is covered by an integral number of units
    (experts can span units on different cores).  Each core gets one X
    unit and one Y unit -> per-core capacity C = SX + SY (1048 for the
    staged routing, vs 1088 for plain expert-per-core), with two weight
    streams per core instead of one.
  - Each core runs a dense FFN over its C tokens in nb blocks (Y split
    into <=512-token blocks + the X block), with D/H features on the
    partition axis end-to-end, so no on-device transposes are needed.
        hT = gelu(w1T.T-contractions)   (PSUM fp32 accum, bias fused in ACT)
        oT = w2-contractions over hT
  - Startup is HBM-limited (all queues share ~350 GB/s): few, large DMAs
    issued before anything else -- x blocks on sync (HWDGE starts ~1us
    before gpsimd's SWDGE), w1Y stream on gpsimd, biases + w1X on scalar.
    PE warm-up matmuls are sized to end right when the first inputs land
    so the HAM clock-gate reaches 8/8 and real matmuls start warm.
  - Host applies combine weights and scatter-adds per-unit outputs back
    into the full [B,S,D] output.

Per-core layouts (D=1024, H=4096, C tokens, P=128):
  xT  [128, 8*C]         bf16   block-major: xT[p, ko*sz+t] = x_blk[t, ko*128+p]
  w1Y/w1X [128, 32, 8, 128] bf16  w1t[p, ho, ko, j] = w1[e][ho*128+j, ko*128+p]
  w2Y/w2X [128, 8, 32, 128] bf16  w2t[p, do, ko, j] = w2[e][do*128+j, ko*128+p]
  b1Y/b1X [128, 32]      f32    b1t[p, ho]         = b1[e][ho*128+p]
  oT  [128, 8, C]        f32    oT[p, do, t]       = o_blk[t, do*128+p]
"""

import numpy as np
import ml_dtypes

TOP_K = 2
P = 128
D = 1024
H = 4096
E = 8
SX = 512
NWARM = 16
WN = 256
LEAD = 3

_COMPILED = {}  # (ybs, sx) -> compiled Bacc instance


def _split_y(SY):
    """Split SY into <=512 blocks (multiples of 8 assumed).  In the
    two-block case b0 takes ~63%: the PE's lead rows on b0 then cover
    the DMA-arrival gap of the second x block."""
    n = -(-SY // 512)
    if n == 2:
        b0 = min(512, ((int(SY * 0.63)) // 8) * 8)
        b1 = SY - b0
        if b1 > 512:
            b1 = 512
            b0 = SY - 512
        return [b0, b1]
    base = ((SY // n + 7) // 8) * 8
    first = SY - base * (n - 1)
    assert 0 < first <= base <= 512, (SY, n, base, first)
    return [first] + [base] * (n - 1)


def _try_assign(cnt, SY):
    """Cover counts with 8 X-units (SX) and 8 Y-units (SY); returns
    [(x_e, y_e)] per expert or None."""
    order = sorted(range(E), key=lambda e: -cnt[e])
    res = [None] * E

    def opts(need, xb, yb):
        out = []
        for y in range(min(yb, 8) + 1):
            rem = need - y * SY
            x = max(0, -(-rem // SX)) if rem > 0 else 0
            if x <= xb:
                tot = x * SX + y * SY
                if tot >= need:
                    out.append((tot - need, x, y))
        out.sort()
        # drop dominated (same or more of both units)
        seen = []
        for w, x, y in out:
            if not any(x >= sx and y >= sy for sx, sy in seen):
                seen.append((x, y))
        return seen

    def dfs(i, xb, yb):
        if i == E:
            return True
        e = order[i]
        for x, y in opts(cnt[e], xb, yb):
            res[e] = (x, y)
            if dfs(i + 1, xb - x, yb - y):
                return True
        res[e] = None
        return False

    return res if dfs(0, 8, 8) else None


def _solve_layout(cnt):
    for SY in range(256, 8 * 4096, 8):
        asn = _try_assign(cnt, SY)
        if asn is not None:
            return SY, asn
    raise RuntimeError("no unit assignment found")


def _build_ffn_kernel(ybs, sx):
    import concourse.mybir as mybir
    import concourse.tile as tile
    from concourse import bacc

    ybs = list(ybs)
    nY = len(ybs)
    blocks = ybs + [sx]          # sizes, processing order
    streams = [0] * nY + [1]     # 0 = Y weights, 1 = X weights
    starts = [sum(blocks[:i]) for i in range(len(blocks))]
    nb = len(blocks)
    C = sum(blocks)
    bf16 = mybir.dt.bfloat16
    f32 = mybir.dt.float32

    nc = bacc.Bacc("TRN2", target_bir_lowering=False, debug=False)
    xT = nc.dram_tensor("xT", [P, D // P * C], bf16, kind="ExternalInput").ap()
    w1Y = nc.dram_tensor("w1Y", [P, H // P, D // P, P], bf16, kind="ExternalInput").ap()
    w1X = nc.dram_tensor("w1X", [P, H // P, D // P, P], bf16, kind="ExternalInput").ap()
    w2Y = nc.dram_tensor("w2Y", [P, D // P, H // P, P], bf16, kind="ExternalInput").ap()
    w2X = nc.dram_tensor("w2X", [P, D // P, H // P, P], bf16, kind="ExternalInput").ap()
    b1Y = nc.dram_tensor("b1Y", [P, H // P], f32, kind="ExternalInput").ap()
    b1X = nc.dram_tensor("b1X", [P, H // P], f32, kind="ExternalInput").ap()
    oT = nc.dram_tensor("oT", [P, D // P, C], f32, kind="ExternalOutput").ap()
    w1d = [w1Y, w1X]
    w2d = [w2Y, w2X]

    with tile.TileContext(nc) as tc:
        with (
            tc.tile_pool(name="const", bufs=1) as cpool,
            tc.tile_pool(name="resident", bufs=1) as rpool,
            tc.tile_pool(name="warm", bufs=1) as warmpool,
            tc.tile_pool(name="w1h", bufs=1) as w1hpool,
            tc.tile_pool(name="w1y", bufs=6) as w1ypool,
            tc.tile_pool(name="w1x", bufs=6) as w1xpool,
            tc.tile_pool(name="w2y", bufs=3) as w2ypool,
            tc.tile_pool(name="w2x", bufs=3) as w2xpool,
            tc.tile_pool(name="ost", bufs=4) as opool,
            tc.tile_pool(name="ps", bufs=4, space="PSUM") as pspool,
            tc.tile_pool(name="wps", bufs=1, space="PSUM") as wpspool,
        ):
            # ---- prologue ----
            # Few, large DMAs: small transfers pay ~2us completion latency
            # each and serialize their queue.  gpsimd streams x (one DMA
            # per block); sync streams w1Y; scalar takes biases + w1X.
            wsrc = warmpool.tile([P, WN], bf16)
            nc.vector.memset(wsrc[:], 0.0)

            # x on sync (HWDGE starts ~1us earlier than gpsimd's SWDGE)
            x_blks = []
            for blk, (st, sz) in enumerate(zip(starts, blocks)):
                xb = rpool.tile([P, D // P * sz], bf16, tag=f"xb{blk}")
                base = D // P * st
                nc.sync.dma_start(xb[:], xT[:, base : base + D // P * sz])
                x_blks.append(xb)

            w1_tiles = [{}, {}]  # stream -> {ho: tile}
            ty0 = w1hpool.tile([P, D // P, P], bf16, tag="w1y_h0")
            nc.gpsimd.dma_start(ty0[:], w1Y[:, 0])
            w1_tiles[0][0] = ty0
            ty1 = w1hpool.tile([P, D // P, P], bf16, tag="w1y_h1")
            nc.gpsimd.dma_start(ty1[:], w1Y[:, 1])
            w1_tiles[0][1] = ty1

            b1y_sb = cpool.tile([P, H // P], f32, tag="b1_0")
            b1x_sb = cpool.tile([P, H // P], f32, tag="b1_1")
            b1sb = [b1y_sb, b1x_sb]
            nc.scalar.dma_start(b1sb[0][:], b1Y[:])
            nc.scalar.dma_start(b1sb[1][:], b1X[:])
            tx = w1hpool.tile([P, D // P, P], bf16, tag="w1x_h0")
            nc.scalar.dma_start(tx[:], w1X[:, 0])
            w1_tiles[1][0] = tx

            # ---- PE warm-up: ends ~when first inputs land ----
            wps = wpspool.tile([P, WN], f32)
            for _ in range(NWARM):
                nc.tensor.matmul(wps[:], wsrc[:, :P], wsrc[:], start=True, stop=True)

            # ---- layer 1 ----
            h_sb = rpool.tile([P, H // P, C], bf16)
            w1pools = [w1ypool, w1xpool]
            w1eng = [nc.gpsimd, nc.scalar]

            def get_w1(s, ho):
                if ho not in w1_tiles[s]:
                    t = w1pools[s].tile([P, D // P, P], bf16, tag=f"w1s{s}")
                    w1eng[s].dma_start(t[:], w1d[s][:, ho])
                    w1_tiles[s][ho] = t
                return w1_tiles[s][ho]

            # lead order tracks DMA arrival: xb0 lands first (b0 rows run
            # while xb1 streams), then b1 rows, then the X block
            pairs = []
            if nb == 3 and LEAD >= 3:
                pairs = [(0, 0), (1, 0), (2, 0), (0, 1), (1, 1), (2, 1),
                         (0, 2), (1, 2), (2, 2)]
                rest = range(3, H // P - 3)
            else:
                for ho in range(LEAD):
                    pairs += [(ho, b) for b in range(nY)]
                for ho in range(LEAD):
                    pairs.append((ho, nY))
                rest = range(LEAD, H // P - 3)
            for ho in rest:
                pairs += [(ho, b) for b in range(nb)]
            # tail rows column-major: b0's h completes 6 groups before the
            # end, so layer 2's first group starts with zero h-wait
            for b in range(nb):
                pairs += [(ho, b) for ho in range(H // P - 3, H // P)]

            for ho, blk in pairs:
                s = streams[blk]
                w1s = get_w1(s, ho)
                st, sz = starts[blk], blocks[blk]
                ps = pspool.tile([P, 512], f32, tag="ps")
                for ko in range(D // P):
                    nc.tensor.matmul(
                        ps[:, :sz],
                        w1s[:, ko, :],
                        x_blks[blk][:, ko * sz : (ko + 1) * sz],
                        start=(ko == 0),
                        stop=(ko == D // P - 1),
                    )
                nc.scalar.activation(
                    h_sb[:, ho, st : st + sz],
                    ps[:, :sz],
                    mybir.ActivationFunctionType.Gelu,
                    bias=b1sb[s][:, ho : ho + 1],
                )

            # ---- layer 2 ----
            w2pools = [w2ypool, w2xpool]
            w2eng = [nc.sync, nc.scalar]  # sync is free once x has landed
            for do in range(D // P):
                w2_t = [None, None]

                def get_w2(s, do=do):
                    if w2_t[s] is None:
                        t = w2pools[s].tile([P, H // P, P], bf16, tag=f"w2s{s}")
                        w2eng[s].dma_start(t[:], w2d[s][:, do])
                        w2_t[s] = t
                    return w2_t[s]

                order = list(range(nb))
                if do == D // P - 1:
                    order = order[::-1]  # finish on the smallest block
                for bi, blk in enumerate(order):
                    s = streams[blk]
                    w2s = get_w2(s)
                    st, sz = starts[blk], blocks[blk]
                    ps = pspool.tile([P, 512], f32, tag="ps")
                    for ko in range(H // P):
                        nc.tensor.matmul(
                            ps[:, :sz],
                            w2s[:, ko, :],
                            h_sb[:, ko, st : st + sz],
                            start=(ko == 0),
                            stop=(ko == H // P - 1),
                        )
                    last = do == D // P - 1 and bi == nb - 1
                    if not last:
                        ob = opool.tile([P, 512], f32, tag="ob")
                        nc.vector.tensor_copy(ob[:, :sz], ps[:, :sz])
                        nc.gpsimd.dma_start(oT[:, do, st : st + sz], ob[:, :sz])
                    else:
                        # final eviction on the critical path: split halves
                        # across two idle queues
                        hsz = sz // 2
                        ob = opool.tile([P, 512], f32, tag="ob")
                        nc.vector.tensor_copy(ob[:, :hsz], ps[:, :hsz])
                        nc.sync.dma_start(oT[:, do, st : st + hsz], ob[:, :hsz])
                        nc.vector.tensor_copy(ob[:, hsz:sz], ps[:, hsz:sz])
                        nc.scalar.dma_start(
                            oT[:, do, st + hsz : st + sz], ob[:, hsz:sz]
                        )

    nc.compile()
    return nc


def _route_host(x_flat, router_w):
    """Float64 router: returns per-expert (token_idx, combine_weight)."""
    logits = x_flat.astype(np.float64) @ router_w.astype(np.float64).T
    m = logits.max(axis=-1, keepdims=True)
    p = np.exp(logits - m)
    p /= p.sum(axis=-1, keepdims=True)
    order = np.argsort(-p, axis=-1)
    topi = order[:, :TOP_K]
    topw = np.take_along_axis(p, topi, axis=-1)
    topw /= topw.sum(axis=-1, keepdims=True)

    idx_list, wgt_list = [], []
    for e in range(E):
        mask = topi == e  # [T, TOP_K]; at most one True per row
        rows = np.nonzero(mask.any(axis=-1))[0]
        w = topw[rows][mask[rows]]
        idx_list.append(rows)
        wgt_list.append(w.astype(np.float32))
    return idx_list, wgt_list


def kernel(x, router_w, w1, b1, w2, b2):
    from concourse import bass_utils

    x = np.asarray(x)
    router_w = np.asarray(router_w)
    w1 = np.asarray(w1)
    b1 = np.asarray(b1)
    w2 = np.asarray(w2)
    b2 = np.asarray(b2)

    B, S, _ = x.shape
    T = B * S
    x_flat = x.reshape(T, D)

    idx_list, wgt_list = _route_host(x_flat, router_w)
    cnt = [len(i) for i in idx_list]
    SY, asn = _solve_layout(cnt)
    ybs = _split_y(SY)
    C = SY + SX

    # units: per expert, y_e Y-units then x_e X-units, consuming its tokens
    # in order; leftover units get expert 0 with zero tokens.
    yunits, xunits = [], []  # (expert, tok_slice)
    for e in range(E):
        xn, yn = asn[e]
        pos = 0
        for _ in range(yn):
            tk = idx_list[e][pos : pos + SY]
            yunits.append((e, pos, pos + len(tk)))
            pos += len(tk)
        for _ in range(xn):
            tk = idx_list[e][pos : pos + SX]
            xunits.append((e, pos, pos + len(tk)))
            pos += len(tk)
        assert pos >= cnt[e]
    while len(yunits) < 8:
        yunits.append((0, 0, 0))
    while len(xunits) < 8:
        xunits.append((0, 0, 0))

    key = (tuple(ybs), SX)
    if key not in _COMPILED:
        _COMPILED[key] = _build_ffn_kernel(*key)
    nc = _COMPILED[key]

    blocks = ybs + [SX]
    starts = [sum(blocks[:i]) for i in range(len(blocks))]
    bf = ml_dtypes.bfloat16

    def pack_w1(e):
        return np.ascontiguousarray(
            w1[e].reshape(H // P, P, D // P, P).transpose(3, 0, 2, 1)
        ).astype(bf)

    def pack_w2(e):
        return np.ascontiguousarray(
            w2[e].reshape(D // P, P, H // P, P).transpose(3, 0, 2, 1)
        ).astype(bf)

    def pack_b1(e):
        return np.ascontiguousarray(b1[e].reshape(H // P, P).T).astype(np.float32)

    w1p, w2p, b1p = {}, {}, {}
    for e in set(u[0] for u in yunits + xunits):
        w1p[e] = pack_w1(e)
        w2p[e] = pack_w2(e)
        b1p[e] = pack_b1(e)

    in_maps = []
    for core in range(E):
        eY, ylo, yhi = yunits[core]
        eX, xlo, xhi = xunits[core]
        xg = np.zeros((C, D), np.float32)
        xg[: yhi - ylo] = x_flat[idx_list[eY][ylo:yhi]]
        xg[SY : SY + xhi - xlo] = x_flat[idx_list[eX][xlo:xhi]]
        xT_full = xg.T.reshape(D // P, P, C).transpose(1, 0, 2)  # [128, 8, C]
        xT_d = np.concatenate(
            [
                xT_full[:, :, st : st + sz].reshape(P, -1)
                for st, sz in zip(starts, blocks)
            ],
            axis=1,
        ).astype(bf)
        in_maps.append(
            {
                "xT": xT_d,
                "w1Y": w1p[eY],
                "w1X": w1p[eX],
                "w2Y": w2p[eY],
                "w2X": w2p[eX],
                "b1Y": b1p[eY],
                "b1X": b1p[eX],
            }
        )

    res = bass_utils.run_bass_kernel_spmd(nc, in_maps, core_ids=list(range(E)))

    out = np.zeros((T, D), np.float32)
    for core in range(E):
        oTc = res.results[core]["oT"]  # [128, 8, C]
        o_g = oTc.transpose(1, 0, 2).reshape(D, C).T  # [C, D]
        eY, ylo, yhi = yunits[core]
        if yhi > ylo:
            idx = idx_list[eY][ylo:yhi]
            out[idx] += wgt_list[eY][ylo:yhi, None] * (
                o_g[: yhi - ylo] + b2[eY][None, :]
            )
        eX, xlo, xhi = xunits[core]
        if xhi > xlo:
            idx = idx_list[eX][xlo:xhi]
            out[idx] += wgt_list[eX][xlo:xhi, None] * (
                o_g[SY : SY + xhi - xlo] + b2[eX][None, :]
            )
    return out.reshape(B, S, D).astype(np.float32)


# revision 29
# speedup vs baseline: 1.0158x; 1.0008x over previous
"""MoE top-2 routing kernel for 8 Trainium2 NeuronCores.

Strategy (expert-parallel with 2-stream load balancing):
  - Host computes the (tiny) router in float64: logits -> softmax -> top-2 ->
    renormalize.  Selection was verified tie-safe: min prob gap between
    2nd/3rd expert is ~8e-6 while cross-backend fp32 logit noise is ~3e-7.
  - The 8192 (token, expert) pairs are packed into 16 capacity units:
    8 "X" units of SX=512 tokens and 8 "Y" units of SY tokens, each unit
    single-expert.  A small solver picks the minimal SY such that every
    expert's token count is covered by an integral number of units
    (experts can span units on different cores).  Each core gets one X
    unit and one Y unit -> per-core capacity C = SX + SY (1048 for the
    staged routing, vs 1088 for plain expert-per-core), with two weight
    streams per core instead of one.
  - Each core runs a dense FFN over its C tokens in nb blocks (Y split
    into <=512-token blocks + the X block), with D/H features on the
    partition axis end-to-end, so no on-device transposes are needed.
        hT = gelu(w1T.T-contractions)   (PSUM fp32 accum, bias fused in ACT)
        oT = w2-contractions over hT
  - Startup is HBM-limited (all queues share ~350 GB/s): few, large DMAs
    issued before anything else -- x blocks on sync (HWDGE starts ~1us
    before gpsimd's SWDGE), w1Y stream on gpsimd, biases + w1X on scalar.
    PE warm-up matmuls are sized to end right when the first inputs land
    so the HAM clock-gate reaches 8/8 and real matmuls start warm.
  - Host applies combine weights and scatter-adds per-unit outputs back
    into the full [B,S,D] output.

Per-core layouts (D=1024, H=4096, C tokens, P=128):
  xT  [128, 8*C]         bf16   block-major: xT[p, ko*sz+t] = x_blk[t, ko*128+p]
  w1Y/w1X [128, 32, 8, 128] bf16  w1t[p, ho, ko, j] = w1[e][ho*128+j, ko*128+p]
  w2Y/w2X [128, 8, 32, 128] bf16  w2t[p, do, ko, j] = w2[e][do*128+j, ko*128+p]
  b1Y/b1X [128, 32]      f32    b1t[p, ho]         = b1[e][ho*128+p]
  oT  [128, 8, C]        f32    oT[p, do, t]       = o_blk[t, do*128+p]
"""

import numpy as np
import ml_dtypes

TOP_K = 2
P = 128
D = 1024
H = 4096
E = 8
SX = 512
NWARM = 30
WN = 256
LEAD = 3

_COMPILED = {}  # (ybs, sx) -> compiled Bacc instance


def _split_y(SY):
    """Split SY into <=512 blocks (multiples of 8 assumed).  In the
    two-block case b0 takes ~63%: the PE's lead rows on b0 then cover
    the DMA-arrival gap of the second x block."""
    n = -(-SY // 512)
    if n == 2:
        b0 = min(512, ((int(SY * 0.63)) // 8) * 8)
        b1 = SY - b0
        if b1 > 512:
            b1 = 512
            b0 = SY - 512
        return [b0, b1]
    base = ((SY // n + 7) // 8) * 8
    first = SY - base * (n - 1)
    assert 0 < first <= base <= 512, (SY, n, base, first)
    return [first] + [base] * (n - 1)


def _try_assign(cnt, SY):
    """Cover counts with 8 X-units (SX) and 8 Y-units (SY); returns
    [(x_e, y_e)] per expert or None."""
    order = sorted(range(E), key=lambda e: -cnt[e])
    res = [None] * E

    def opts(need, xb, yb):
        out = []
        for y in range(min(yb, 8) + 1):
            rem = need - y * SY
            x = max(0, -(-rem // SX)) if rem > 0 else 0
            if x <= xb:
                tot = x * SX + y * SY
                if tot >= need:
                    out.append((tot - need, x, y))
        out.sort()
        # drop dominated (same or more of both units)
        seen = []
        for w, x, y in out:
            if not any(x >= sx and y >= sy for sx, sy in seen):
                seen.append((x, y))
        return seen

    def dfs(i, xb, yb):
        if i == E:
            return True
        e = order[i]
        for x, y in opts(cnt[e], xb, yb):
            res[e] = (x, y)
            if dfs(i + 1, xb - x, yb - y):
                return True
        res[e] = None
        return False

    return res if dfs(0, 8, 8) else None


def _solve_layout(cnt):
    for SY in range(256, 8 * 4096, 8):
        asn = _try_assign(cnt, SY)
        if asn is not None:
            return SY, asn
    raise RuntimeError("no unit assignment found")


def _build_ffn_kernel(ybs, sx):
    import concourse.mybir as mybir
    import concourse.tile as tile
    from concourse import bacc

    ybs = list(ybs)
    nY = len(ybs)
    blocks = ybs + [sx]          # sizes, processing order
    streams = [0] * nY + [1]     # 0 = Y weights, 1 = X weights
    starts = [sum(blocks[:i]) for i in range(len(blocks))]
    nb = len(blocks)
    C = sum(blocks)
    bf16 = mybir.dt.bfloat16
    f32 = mybir.dt.float32

    nc = bacc.Bacc("TRN2", target_bir_lowering=False, debug=False)
    xT = nc.dram_tensor("xT", [P, D // P * C], bf16, kind="ExternalInput").ap()
    w1Y = nc.dram_tensor("w1Y", [P, H // P, D // P, P], bf16, kind="ExternalInput").ap()
    w1X = nc.dram_tensor("w1X", [P, H // P, D // P, P], bf16, kind="ExternalInput").ap()
    w2Y = nc.dram_tensor("w2Y", [P, D // P, H // P, P], bf16, kind="ExternalInput").ap()
    w2X = nc.dram_tensor("w2X", [P, D // P, H // P, P], bf16, kind="ExternalInput").ap()
    b1Y = nc.dram_tensor("b1Y", [P, H // P], f32, kind="ExternalInput").ap()
    b1X = nc.dram_tensor("b1X", [P, H // P], f32, kind="ExternalInput").ap()
    oT = nc.dram_tensor("oT", [P, D // P, C], f32, kind="ExternalOutput").ap()
    w1d = [w1Y, w1X]
    w2d = [w2Y, w2X]

    with tile.TileContext(nc) as tc:
        with (
            tc.tile_pool(name="const", bufs=1) as cpool,
            tc.tile_pool(name="resident", bufs=1) as rpool,
            tc.tile_pool(name="warm", bufs=1) as warmpool,
            tc.tile_pool(name="w1h", bufs=1) as w1hpool,
            tc.tile_pool(name="w1y", bufs=6) as w1ypool,
            tc.tile_pool(name="w1x", bufs=6) as w1xpool,
            tc.tile_pool(name="w2y", bufs=3) as w2ypool,
            tc.tile_pool(name="w2x", bufs=3) as w2xpool,
            tc.tile_pool(name="ost", bufs=4) as opool,
            tc.tile_pool(name="ps", bufs=4, space="PSUM") as pspool,
            tc.tile_pool(name="wps", bufs=1, space="PSUM") as wpspool,
        ):
            # ---- prologue ----
            # Few, large DMAs: small transfers pay ~2us completion latency
            # each and serialize their queue.  gpsimd streams x (one DMA
            # per block); sync streams w1Y; scalar takes biases + w1X.
            wsrc = warmpool.tile([P, WN], bf16)
            nc.vector.memset(wsrc[:], 0.0)

            # x on sync (HWDGE starts ~1us earlier than gpsimd's SWDGE)
            x_blks = []
            for blk, (st, sz) in enumerate(zip(starts, blocks)):
                xb = rpool.tile([P, D // P * sz], bf16, tag=f"xb{blk}")
                base = D // P * st
                nc.sync.dma_start(xb[:], xT[:, base : base + D // P * sz])
                x_blks.append(xb)

            w1_tiles = [{}, {}]  # stream -> {ho: tile}
            ty0 = w1hpool.tile([P, D // P, P], bf16, tag="w1y_h0")
            nc.gpsimd.dma_start(ty0[:], w1Y[:, 0])
            w1_tiles[0][0] = ty0
            ty1 = w1hpool.tile([P, D // P, P], bf16, tag="w1y_h1")
            nc.gpsimd.dma_start(ty1[:], w1Y[:, 1])
            w1_tiles[0][1] = ty1

            b1y_sb = cpool.tile([P, H // P], f32, tag="b1_0")
            b1x_sb = cpool.tile([P, H // P], f32, tag="b1_1")
            b1sb = [b1y_sb, b1x_sb]
            nc.scalar.dma_start(b1sb[0][:], b1Y[:])
            nc.scalar.dma_start(b1sb[1][:], b1X[:])
            tx = w1hpool.tile([P, D // P, P], bf16, tag="w1x_h0")
            nc.scalar.dma_start(tx[:], w1X[:, 0])
            w1_tiles[1][0] = tx

            # ---- PE warm-up: ends ~when first inputs land ----
            wps = wpspool.tile([P, WN], f32)
            for _ in range(NWARM):
                nc.tensor.matmul(wps[:], wsrc[:, :P], wsrc[:], start=True, stop=True)

            # ---- layer 1 ----
            h_sb = rpool.tile([P, H // P, C], bf16)
            w1pools = [w1ypool, w1xpool]
            w1eng = [nc.gpsimd, nc.scalar]

            def get_w1(s, ho):
                if ho not in w1_tiles[s]:
                    t = w1pools[s].tile([P, D // P, P], bf16, tag=f"w1s{s}")
                    w1eng[s].dma_start(t[:], w1d[s][:, ho])
                    w1_tiles[s][ho] = t
                return w1_tiles[s][ho]

            # lead order tracks DMA arrival: xb0 lands first (b0 rows run
            # while xb1 streams), then b1 rows, then the X block
            pairs = []
            if nb == 3 and LEAD >= 3:
                pairs = [(0, 0), (1, 0), (2, 0), (0, 1), (1, 1), (2, 1),
                         (0, 2), (1, 2), (2, 2)]
                rest = range(3, H // P - 3)
            else:
                for ho in range(LEAD):
                    pairs += [(ho, b) for b in range(nY)]
                for ho in range(LEAD):
                    pairs.append((ho, nY))
                rest = range(LEAD, H // P - 3)
            for ho in rest:
                pairs += [(ho, b) for b in range(nb)]
            # tail rows column-major: b0's h completes 6 groups before the
            # end, so layer 2's first group starts with zero h-wait
            for b in range(nb):
                pairs += [(ho, b) for ho in range(H // P - 3, H // P)]

            for ho, blk in pairs:
                s = streams[blk]
                w1s = get_w1(s, ho)
                st, sz = starts[blk], blocks[blk]
                ps = pspool.tile([P, 512], f32, tag="ps")
                for ko in range(D // P):
                    nc.tensor.matmul(
                        ps[:, :sz],
                        w1s[:, ko, :],
                        x_blks[blk][:, ko * sz : (ko + 1) * sz],
                        start=(ko == 0),
                        stop=(ko == D // P - 1),
                    )
                nc.scalar.activation(
                    h_sb[:, ho, st : st + sz],
                    ps[:, :sz],
                    mybir.ActivationFunctionType.Gelu,
                    bias=b1sb[s][:, ho : ho + 1],
                )

            # ---- layer 2 ----
            w2pools = [w2ypool, w2xpool]
            w2eng = [nc.sync, nc.scalar]  # sync is free once x has landed
            for do in range(D // P):
                w2_t = [None, None]

                def get_w2(s, do=do):
                    if w2_t[s] is None:
                        t = w2pools[s].tile([P, H // P, P], bf16, tag=f"w2s{s}")
                        w2eng[s].dma_start(t[:], w2d[s][:, do])
                        w2_t[s] = t
                    return w2_t[s]

                order = list(range(nb))
                if do == D // P - 1:
                    order = order[::-1]  # finish on the smallest block
                for bi, blk in enumerate(order):
                    s = streams[blk]
                    w2s = get_w2(s)
                    st, sz = starts[blk], blocks[blk]
                    ps = pspool.tile([P, 512], f32, tag="ps")
                    for ko in range(H // P):
                        nc.tensor.matmul(
                            ps[:, :sz],
                            w2s[:, ko, :],
                            h_sb[:, ko, st : st + sz],
                            start=(ko == 0),
                            stop=(ko == H // P - 1),
                        )
                    last = do == D // P - 1 and bi == nb - 1
                    if not last:
                        ob = opool.tile([P, 512], f32, tag="ob")
                        nc.vector.tensor_copy(ob[:, :sz], ps[:, :sz])
                        nc.gpsimd.dma_start(oT[:, do, st : st + sz], ob[:, :sz])
                    else:
                        # final eviction on the critical path: split halves
                        # across two idle queues
                        hsz = sz // 2
                        ob = opool.tile([P, 512], f32, tag="ob")
                        nc.vector.tensor_copy(ob[:, :hsz], ps[:, :hsz])
                        nc.sync.dma_start(oT[:, do, st : st + hsz], ob[:, :hsz])
                        nc.vector.tensor_copy(ob[:, hsz:sz], ps[:, hsz:sz])
                        nc.scalar.dma_start(
                            oT[:, do, st + hsz : st + sz], ob[:, hsz:sz]
                        )

    nc.compile()
    return nc


def _route_host(x_flat, router_w):
    """Float64 router: returns per-expert (token_idx, combine_weight)."""
    logits = x_flat.astype(np.float64) @ router_w.astype(np.float64).T
    m = logits.max(axis=-1, keepdims=True)
    p = np.exp(logits - m)
    p /= p.sum(axis=-1, keepdims=True)
    order = np.argsort(-p, axis=-1)
    topi = order[:, :TOP_K]
    topw = np.take_along_axis(p, topi, axis=-1)
    topw /= topw.sum(axis=-1, keepdims=True)

    idx_list, wgt_list = [], []
    for e in range(E):
        mask = topi == e  # [T, TOP_K]; at most one True per row
        rows = np.nonzero(mask.any(axis=-1))[0]
        w = topw[rows][mask[rows]]
        idx_list.append(rows)
        wgt_list.append(w.astype(np.float32))
    return idx_list, wgt_list


def kernel(x, router_w, w1, b1, w2, b2):
    from concourse import bass_utils

    x = np.asarray(x)
    router_w = np.asarray(router_w)
    w1 = np.asarray(w1)
    b1 = np.asarray(b1)
    w2 = np.asarray(w2)
    b2 = np.asarray(b2)

    B, S, _ = x.shape
    T = B * S
    x_flat = x.reshape(T, D)

    idx_list, wgt_list = _route_host(x_flat, router_w)
    cnt = [len(i) for i in idx_list]
    SY, asn = _solve_layout(cnt)
    ybs = _split_y(SY)
    C = SY + SX

    # units: per expert, y_e Y-units then x_e X-units, consuming its tokens
    # in order; leftover units get expert 0 with zero tokens.
    yunits, xunits = [], []  # (expert, tok_slice)
    for e in range(E):
        xn, yn = asn[e]
        pos = 0
        for _ in range(yn):
            tk = idx_list[e][pos : pos + SY]
            yunits.append((e, pos, pos + len(tk)))
            pos += len(tk)
        for _ in range(xn):
            tk = idx_list[e][pos : pos + SX]
            xunits.append((e, pos, pos + len(tk)))
            pos += len(tk)
        assert pos >= cnt[e]
    while len(yunits) < 8:
        yunits.append((0, 0, 0))
    while len(xunits) < 8:
        xunits.append((0, 0, 0))

    key = (tuple(ybs), SX)
    if key not in _COMPILED:
        _COMPILED[key] = _build_ffn_kernel(*key)
    nc = _COMPILED[key]

    blocks = ybs + [SX]
    starts = [sum(blocks[:i]) for i in range(len(blocks))]
    bf = ml_dtypes.bfloat16

    def pack_w1(e):
        return np.ascontiguousarray(
            w1[e].reshape(H // P, P, D // P, P).transpose(3, 0, 2, 1)
        ).astype(bf)

    def pack_w2(e):
        return np.ascontiguousarray(
            w2[e].reshape(D // P, P, H // P, P).transpose(3, 0, 2, 1)
        ).astype(bf)

    def pack_b1(e):
        return np.ascontiguousarray(b1[e].reshape(H // P, P).T).astype(np.float32)

    w1p, w2p, b1p = {}, {}, {}
    for e in set(u[0] for u in yunits + xunits):
        w1p[e] = pack_w1(e)
        w2p[e] = pack_w2(e)
        b1p[e] = pack_b1(e)

    in_maps = []
    for core in range(E):
        eY, ylo, yhi = yunits[core]
        eX, xlo, xhi = xunits[core]
        xg = np.zeros((C, D), np.float32)
        xg[: yhi - ylo] = x_flat[idx_list[eY][ylo:yhi]]
        xg[SY : SY + xhi - xlo] = x_flat[idx_list[eX][xlo:xhi]]
        xT_full = xg.T.reshape(D // P, P, C).transpose(1, 0, 2)  # [128, 8, C]
        xT_d = np.concatenate(
            [
                xT_full[:, :, st : st + sz].reshape(P, -1)
                for st, sz in zip(starts, blocks)
            ],
            axis=1,
        ).astype(bf)
        in_maps.append(
            {
                "xT": xT_d,
                "w1Y": w1p[eY],
                "w1X": w1p[eX],
                "w2Y": w2p[eY],
                "w2X": w2p[eX],
                "b1Y": b1p[eY],
                "b1X": b1p[eX],
            }
        )

    res = bass_utils.run_bass_kernel_spmd(nc, in_maps, core_ids=list(range(E)))

    out = np.zeros((T, D), np.float32)
    for core in range(E):
        oTc = res.results[core]["oT"]  # [128, 8, C]
        o_g = oTc.transpose(1, 0, 2).reshape(D, C).T  # [C, D]
        eY, ylo, yhi = yunits[core]
        if yhi > ylo:
            idx = idx_list[eY][ylo:yhi]
            out[idx] += wgt_list[eY][ylo:yhi, None] * (
                o_g[: yhi - ylo] + b2[eY][None, :]
            )
        eX, xlo, xhi = xunits[core]
        if xhi > xlo:
            idx = idx_list[eX][xlo:xhi]
            out[idx] += wgt_list[eX][xlo:xhi, None] * (
                o_g[SY : SY + xhi - xlo] + b2[eX][None, :]
            )
    return out.reshape(B, S, D).astype(np.float32)
